# revision 16
# baseline (speedup 1.0000x reference)
"""Trainium2 Bass kernel for the EvolutionBank scatter+temporal-consistency op.

Math per selected row i (idx unique):
    p = ptr[idx[i]] % 6
    window = bank[idx[i]]            # (6, 32)
    window[p] = emb[i]               # circular-buffer write
    v_w = window / max(||window||, eps)
    sim_q = <v_q, v_{q+1}>,  q = 0..4
    out[i] = 1 / (1 + std(sim, ddof=1))

Distribution: the B=200k referenced rows are sharded across 8 cores. On
the host, each core's 25k rows are routed into 6 buckets by their write
slot p (expert-parallel routing, padded to a fixed 4608 capacity), so
each device tile has a *static* replaced slot: the scatter becomes a
static slot substitution in the access patterns. One tile per bucket
(128 partitions x 36 rows); per tile a combined (rows, 11, 32) tensor
holds the 6 squared slots + 5 adjacent products of the merged window,
and two segmented reduces yield all 11 dots per row.

Raw Bass with manual semaphores (the Tile layer emits >2 sync waits per
instruction, which this walrus rejects; standalone wait_ge instructions
have no such cap). Engine split / software pipeline:
    SP   : HWDGE loads + stores
    ACT  : squares of the merged window; the two small sqrts (norm
           products, std) for earlier tiles interleaved between tiles
    POOL : the 5 adjacent products (gpsimd tensor_tensor), or DVE
    DVE  : segmented reduces + the whole consistency tail
"""

import os
import sys

for _p in ("/opt/trn_rl_repo", os.path.expanduser("~/.axon_site/_ro/trn_rl_repo")):
    if os.path.isdir(_p) and _p not in sys.path:
        sys.path.insert(0, _p)

import numpy as np

NUM_NODES = 1_000_000
W = 6
D = 32
B = 200_000
NCORES = 8
PER = B // NCORES            # 25000 rows per core
RPP = 36                     # rows per partition per tile
CAP = 128 * RPP              # 4608 padded bucket capacity (max bucket ~4350)
NT = W                       # one tile per bucket
EPS = 1e-6

USE_GPSIMD_PRODUCTS = os.environ.get("EVO_GPSIMD", "1") == "1"
N_RUNS = int(os.environ.get("EVO_RUNS", "2"))  # >=2: first run is warmup

_prog = None
LAST_RESULTS = None


def _build(reps=1):
    global _prog
    if reps == 1 and _prog is not None:
        return _prog

    from contextlib import ExitStack

    import concourse.bass as bass
    from concourse import mybir

    f32 = mybir.dt.float32
    X = mybir.AxisListType.X
    AF = mybir.ActivationFunctionType
    MUL = mybir.AluOpType.mult

    nc = bass.Bass(
        detect_race_conditions=os.environ.get("EVO_RACE_DETECT", "0") == "1"
    )
    bank_h = nc.declare_dram_parameter("bank", [NT, 128, RPP, W, D], f32, isOutput=False)
    emb_h = nc.declare_dram_parameter("emb", [NT, 128, RPP, 1, D], f32, isOutput=False)
    out_h = nc.declare_dram_parameter("out", [NT, 128, RPP], f32, isOutput=True)

    with ExitStack() as ctx:
        block = ctx.enter_context(nc.Block())
        sb = lambda name, shape, dt=f32: ctx.enter_context(
            nc.sbuf_tensor(name, shape, dt)
        )
        sem = lambda name: ctx.enter_context(nc.semaphore(name))

        bank_sb = sb("bank_sb", [128, 2, RPP, W, D])
        emb_sb = sb("emb_sb", [128, 2, RPP, 1, D])
        comb_sb = sb("comb_sb", [128, 2, RPP, 2 * W - 1, D])
        red_sb = sb("red_sb", [128, 2, RPP, 2 * W - 1])
        nmax_sb = sb("nmax_sb", [128, 2, RPP, W])
        den2_sb = sb("den2_sb", [128, 2, RPP * (W - 1)])
        nd_sb = sb("nd_sb", [128, 2, RPP * (W - 1)])
        rec_sb = sb("rec_sb", [128, 2, RPP * (W - 1)])
        sim_sb = sb("sim_sb", [128, 2, RPP, W - 1])
        simsq_sb = sb("simsq_sb", [128, 2, RPP, W - 1])
        s1_sb = sb("s1_sb", [128, 2, RPP])
        s2_sb = sb("s2_sb", [128, 2, RPP])
        s1h_sb = sb("s1h_sb", [128, 2, RPP])
        s1sq_sb = sb("s1sq_sb", [128, 2, RPP])
        var4_sb = sb("var4_sb", [128, 2, RPP])
        varc_sb = sb("varc_sb", [128, 2, RPP])
        stdt_sb = sb("stdt_sb", [128, 2, RPP])
        u_sb = sb("u_sb", [128, 2, RPP])
        cons_sb = sb("cons_sb", [128, 2, RPP])

        # DMA completions are unordered across transfers, so completion
        # sems are split by buffer parity: issue-gating guarantees at most
        # one outstanding same-parity transfer per stream.
        ld_b = [sem("ld_b0"), sem("ld_b1")]  # bank loads, +16 each
        ld_e = [sem("ld_e0"), sem("ld_e1")]  # emb loads, +16 each
        st2 = [sem("st0"), sem("st1")]       # stores, +16 each
        act_sq = sem("act_sq")  # +1 per tile: squares done
        act_s1 = sem("act_s1")  # +1 per tile: sqrt(den2) done
        act_s2 = sem("act_s2")  # +1 per tile: sqrt(varc) done
        dve_a = sem("dve_a")    # +1 per tile: reduces/nmax/den2 done
        dve_b = sem("dve_b")    # +1 per tile: sim/var done
        dve_c = sem("dve_c")    # +1 per tile: cons done
        pool_p = sem("pool_p")  # +1 per tile: products done

        TOT = NT * reps

        @block.sync
        def _(sync):
            for i in range(TOT):
                s = i % 2
                if i >= 2:
                    # bank/emb slot s free: squares + products of tile i-2 done
                    sync.wait_ge(act_sq, i - 1)
                    sync.wait_ge(pool_p, i - 1)
                sync.dma_start(out=bank_sb[:, s], in_=bank_h[i % NT]).then_inc(
                    ld_b[s], 16
                )
                sync.dma_start(out=emb_sb[:, s], in_=emb_h[i % NT]).then_inc(
                    ld_e[s], 16
                )
                if i >= 2:
                    # stores lag loads by 2 tiles (C_j completes at DVE
                    # pipeline step j+2, so an earlier store wait deadlocks)
                    sync.wait_ge(dve_c, i - 1)
                    sync.dma_start(
                        out=out_h[(i - 2) % NT], in_=cons_sb[:, (i - 2) % 2]
                    ).then_inc(st2[(i - 2) % 2], 16)
            for j in (TOT - 2, TOT - 1):
                sync.wait_ge(dve_c, j + 1)
                sync.dma_start(out=out_h[j % NT], in_=cons_sb[:, j % 2]).then_inc(
                    st2[j % 2], 16
                )
            sync.wait_ge(st2[0], 16 * ((TOT + 1) // 2))
            sync.wait_ge(st2[1], 16 * (TOT // 2))

        def act_squares(scalar, i):
            s = i % 2
            scalar.wait_ge(ld_b[s], 16 * (i // 2 + 1))
            scalar.wait_ge(ld_e[s], 16 * (i // 2 + 1))
            if i >= 2:
                scalar.wait_ge(dve_a, i - 1)  # comb slot s free
            w = i % NT  # bucket index
            if w > 0:
                scalar.square(
                    comb_sb[:, s, :, 0:w, :], bank_sb[:, s, :, 0:w, :]
                )
            if w < W - 1:
                scalar.square(
                    comb_sb[:, s, :, w + 1 : W, :], bank_sb[:, s, :, w + 1 : W, :]
                )
            scalar.square(comb_sb[:, s, :, w : w + 1, :], emb_sb[:, s]).then_inc(
                act_sq, 1
            )

        def act_q1(scalar, j):  # nd = sqrt(den2) for tile j
            scalar.wait_ge(dve_a, j + 1)
            if j >= 2:
                scalar.wait_ge(dve_b, j - 1)  # nd slot free (B_{j-2} done)
            scalar.sqrt(nd_sb[:, j % 2], den2_sb[:, j % 2]).then_inc(act_s1, 1)

        def act_q2(scalar, j):  # stdt = sqrt(varc) for tile j
            scalar.wait_ge(dve_b, j + 1)
            if j >= 2:
                scalar.wait_ge(dve_c, j - 1)  # stdt slot free (C_{j-2} done)
            scalar.sqrt(stdt_sb[:, j % 2], varc_sb[:, j % 2]).then_inc(act_s2, 1)

        @block.scalar
        def _(scalar):
            # software pipeline: SQ_i | Q1_{i-1} | Q2_{i-2}
            for i in range(TOT + 2):
                if i < TOT:
                    act_squares(scalar, i)
                if 1 <= i <= TOT:
                    act_q1(scalar, i - 1)
                if i >= 2:
                    act_q2(scalar, i - 2)

        def prod_ops(eng, i):
            s = i % 2
            eng.wait_ge(ld_b[s], 16 * (i // 2 + 1))
            eng.wait_ge(ld_e[s], 16 * (i // 2 + 1))
            if i >= 2:
                eng.wait_ge(dve_a, i - 1)  # comb slot s free
            w = i % NT
            last = None
            if w >= 2:
                last = eng.tensor_mul(
                    comb_sb[:, s, :, W : W + w - 1, :],
                    bank_sb[:, s, :, 0 : w - 1, :],
                    bank_sb[:, s, :, 1:w, :],
                )
            if w <= W - 3:
                last = eng.tensor_mul(
                    comb_sb[:, s, :, W + w + 1 : 2 * W - 1, :],
                    bank_sb[:, s, :, w + 1 : W - 1, :],
                    bank_sb[:, s, :, w + 2 : W, :],
                )
            if w >= 1:
                last = eng.tensor_mul(
                    comb_sb[:, s, :, W + w - 1 : W + w, :],
                    bank_sb[:, s, :, w - 1 : w, :],
                    emb_sb[:, s],
                )
            if w <= W - 2:
                last = eng.tensor_mul(
                    comb_sb[:, s, :, W + w : W + w + 1, :],
                    emb_sb[:, s],
                    bank_sb[:, s, :, w + 1 : w + 2, :],
                )
            last.then_inc(pool_p, 1)

        if USE_GPSIMD_PRODUCTS:

            @block.gpsimd
            def _(gpsimd):
                for i in range(TOT):
                    prod_ops(gpsimd, i)

        # Same-engine RAW interlock: consecutive DVE ops can read stale SBUF
        # (the write of a small preceding op may not have landed). Every DVE
        # op incs dve_self; a dependent op first waits for the full count.
        dve_self = sem("dve_self")
        dve_cnt = [0]

        def dvi(ins):
            ins.then_inc(dve_self, 1)
            dve_cnt[0] += 1
            return ins

        def dviw(vector):
            if dve_cnt[0]:
                vector.wait_ge(dve_self, dve_cnt[0])

        def dve_stage_a(vector, i):
            s = i % 2
            vector.wait_ge(act_sq, i + 1)
            vector.wait_ge(pool_p, i + 1)
            if i >= 2:
                vector.wait_ge(act_s1, i - 1)  # den2 slot free (Q1_{i-2} done)
            dvi(
                vector.reduce_sum(
                    red_sb[:, s, :, 0:W], comb_sb[:, s, :, 0:W, :], axis=X
                )
            )
            dvi(
                vector.reduce_sum(
                    red_sb[:, s, :, W : 2 * W - 1],
                    comb_sb[:, s, :, W : 2 * W - 1, :],
                    axis=X,
                )
            )
            dviw(vector)
            dvi(
                vector.tensor_scalar_max(
                    nmax_sb[:, s], red_sb[:, s, :, 0:W], EPS * EPS
                )
            )
            den2_3d = den2_sb[:, s].rearrange("p (r q) -> p r q", q=W - 1)
            dviw(vector)
            vector.tensor_mul(
                den2_3d, nmax_sb[:, s, :, 0 : W - 1], nmax_sb[:, s, :, 1:W]
            ).then_inc(dve_a, 1)

        def dve_stage_b(vector, i):
            s = i % 2
            vector.wait_ge(act_s1, i + 1)
            if i >= 2:
                vector.wait_ge(act_s2, i - 1)  # varc slot free (Q2_{i-2} done)
            dvi(vector.reciprocal(out=rec_sb[:, s], in_=nd_sb[:, s]))
            rec_3d = rec_sb[:, s].rearrange("p (r q) -> p r q", q=W - 1)
            dviw(vector)
            dvi(
                vector.tensor_mul(
                    sim_sb[:, s], red_sb[:, s, :, W : 2 * W - 1], rec_3d
                )
            )
            dviw(vector)
            dvi(vector.reduce_sum(s1_sb[:, s], sim_sb[:, s], axis=X))
            dvi(vector.tensor_mul(simsq_sb[:, s], sim_sb[:, s], sim_sb[:, s]))
            dviw(vector)
            dvi(vector.reduce_sum(s2_sb[:, s], simsq_sb[:, s], axis=X))
            dviw(vector)
            dvi(
                vector.scalar_tensor_tensor(
                    out=s1sq_sb[:, s],
                    in0=s1_sb[:, s],
                    scalar=0.05,
                    in1=s1_sb[:, s],
                    op0=MUL,
                    op1=MUL,
                )
            )
            dviw(vector)
            dvi(
                vector.scalar_tensor_tensor(
                    out=var4_sb[:, s],
                    in0=s2_sb[:, s],
                    scalar=0.25,
                    in1=s1sq_sb[:, s],
                    op0=MUL,
                    op1=mybir.AluOpType.subtract,
                )
            )
            dviw(vector)
            vector.tensor_scalar_max(varc_sb[:, s], var4_sb[:, s], 0.0).then_inc(
                dve_b, 1
            )

        def dve_stage_c(vector, i):
            s = i % 2
            vector.wait_ge(act_s2, i + 1)
            if i >= 2:
                vector.wait_ge(st2[s], 16 * (i // 2))  # cons slot s free
            dvi(vector.tensor_scalar_add(u_sb[:, s], stdt_sb[:, s], 1.0))
            dviw(vector)
            vector.reciprocal(out=cons_sb[:, s], in_=u_sb[:, s]).then_inc(dve_c, 1)

        @block.vector
        def _(vector):
            if not USE_GPSIMD_PRODUCTS:
                # products on DVE, same A/B/C pipeline
                for i in range(TOT + 2):
                    if i < TOT:
                        prod_ops(vector, i)
                        dve_stage_a(vector, i)
                    if 1 <= i <= TOT:
                        dve_stage_b(vector, i - 1)
                    if i >= 2:
                        dve_stage_c(vector, i - 2)
            else:
                # software pipeline: A_i | B_{i-1} | C_{i-2}
                for i in range(TOT + 2):
                    if i < TOT:
                        dve_stage_a(vector, i)
                    if 1 <= i <= TOT:
                        dve_stage_b(vector, i - 1)
                    if i >= 2:
                        dve_stage_c(vector, i - 2)

    if reps == 1:
        _prog = nc
    return nc


def kernel(bank, emb, idx, ptr, filled=None, **_unused):
    global LAST_RESULTS
    from concourse.bass_utils import run_bass_kernel_spmd

    nc = _build()

    bank = np.asarray(bank)
    emb = np.asarray(emb, dtype=np.float32)
    idx_i = np.asarray(idx).astype(np.int64)
    ptr_i = np.asarray(ptr).astype(np.int64)

    assert bank.shape == (NUM_NODES, W, D) and emb.shape == (B, D)
    bank2 = np.ascontiguousarray(bank.astype(np.float32, copy=False)).reshape(
        NUM_NODES, W * D
    )
    p_all = (ptr_i[idx_i] % W).astype(np.int64)  # (B,) write slot per selected row

    in_maps = []
    metas = []
    for c in range(NCORES):
        sl = slice(c * PER, (c + 1) * PER)
        pc = p_all[sl]
        counts = np.bincount(pc, minlength=W)
        assert counts.max() <= CAP, f"bucket overflow: {counts}"
        order = np.argsort(pc, kind="stable")
        starts = np.zeros(W + 1, np.int64)
        starts[1:] = np.cumsum(counts)
        slot_rows = np.zeros(W * CAP, dtype=np.int64)
        for w in range(W):
            seg = order[starts[w] : starts[w + 1]]
            slot_rows[w * CAP : w * CAP + counts[w]] = seg
            pad_src = seg[0] if counts[w] > 0 else 0
            slot_rows[w * CAP + counts[w] : (w + 1) * CAP] = pad_src

        g_rows = idx_i[sl][slot_rows]  # global bank row per padded slot
        bank_c = bank2[g_rows]  # (W*CAP, 192)
        emb_c = emb[sl][slot_rows]  # (W*CAP, 32)
        in_maps.append(
            {
                "bank": np.ascontiguousarray(bank_c).reshape(NT, 128, RPP, W, D),
                "emb": np.ascontiguousarray(emb_c).reshape(NT, 128, RPP, 1, D),
            }
        )
        metas.append((slot_rows, counts))

    trace = os.environ.get("EVO_TRACE", "0") == "1"
    res = None
    for _ in range(max(1, N_RUNS)):
        res = run_bass_kernel_spmd(nc, in_maps, list(range(NCORES)), trace=trace)
    LAST_RESULTS = res

    out = np.empty(B, dtype=np.float32)
    for c in range(NCORES):
        cons = np.asarray(res.results[c]["out"]).reshape(W * CAP)
        slot_rows, counts = metas[c]
        for w in range(W):
            n = counts[w]
            out[c * PER + slot_rows[w * CAP : w * CAP + n]] = cons[
                w * CAP : w * CAP + n
            ]
    return out
